# revision 1
# baseline (speedup 1.0000x reference)
"""Trainium2 Bass kernel for a single causal attention head.

Problem: x:(8,2048,1024) f32, per-head projections wq/wk/wv:(64,1024),
biases (64,). Output: softmax(causal(q k^T / sqrt(64))) @ v : (8,2048,64).

Strategy:
  - Data-parallel: batch b -> core b (8 cores, 1 batch each).
  - Host prep: x[b] transposed to xT:(1024,2048) fp16 (contraction dim D on
    SBUF partitions); Q/K weights shipped stacked as [wq|wk] (fp16, 1/sqrt(64)
    folded into wq).
  - Device (per core):
      * qk1 = [wq|wk]^T.T @ xT: rows 0-63 = Q^T, rows 64-127 = K^T (PSUM
        accumulate over 8 d-tiles, fp16 matmuls, N=512 chunks).
      * qk2 = half-swapped copy of qk1 (SBUF->SBUF DMA): K^T on rows 0-63,
        Q^T on rows 64-127. Both copies exist on both partition halves ->
        scores for TWO k-tiles run concurrently via PE row packing.
      * vT transposed back to (T,64) tiles via PE transpose, augmented with a
        ones column (softmax denominator rides along the PV matmul).
      * S^T[j,i] = sum_h K^T[h,j] Q^T[h,i] transposed-scores layout; P^T =
        exp(S^T) on ACT, one [128,1024] instr per k-tile pair; causal mask =
        GPSIMD affine_select zeroing P^T above the diagonal (identical
        result: zeros add nothing to numerator or denominator).
      * O^T_aug[65, T] accumulated in PSUM over k-tiles; row 64 = sum_j P^T.
      * causal skip: k-tiles entirely above the diagonal never computed.
      * attention for chunk ci is emitted right after projection chunk ci, so
        exp/PV overlap later projections instead of serializing at the end.
  - Host post: out[b] = (O^T[0:64] / O^T[64:65]).T  (softmax normalization).
"""

import numpy as np

B, T, D, HD = 8, 2048, 1024, 64
P = 128          # SBUF partitions
CH = 512         # q-chunk (matmul moving dim)
NCH = T // CH    # 4
DT = D // P      # 8 d-tiles
NKT = T // P     # 16 k-tiles
HT = T // 2      # xT half-tile width

LAST_RESULTS = None      # BassKernelResults of the most recent run (for test.py)


def _build_module(legalize=True):
    import concourse.bass as bass
    import concourse.mybir as mybir
    from concourse.tile import TileContext

    from concourse.masks import make_identity
    F32 = mybir.dt.float32
    F16 = mybir.dt.float16

    nc = bass.Bass("TRN2", target_bir_lowering=True)

    xT = nc.dram_tensor("xT", (D, T), F16, kind="ExternalInput")
    w1 = nc.dram_tensor("w1", (D, P), F16, kind="ExternalInput")   # [wq*s | wk]^T
    wv = nc.dram_tensor("wv", (D, HD), F16, kind="ExternalInput")  # wv^T
    b1 = nc.dram_tensor("b1", (P, 1), F32, kind="ExternalInput")   # [bq*s; bk]
    bv = nc.dram_tensor("bv", (P, 1), F32, kind="ExternalInput")  # [bv; bv]
    outT = nc.dram_tensor("outT", (HD + 1, T), F32, kind="ExternalOutput")

    with TileContext(nc) as tc:
        with (
            tc.tile_pool(name="const", bufs=1) as const,
            tc.tile_pool(name="acts", bufs=1) as acts,
            tc.tile_pool(name="proj_ps", bufs=2, space="PSUM") as proj_ps,
            tc.tile_pool(name="tr_ps", bufs=1, space="PSUM") as tr_ps,
            tc.tile_pool(name="s_ps", bufs=2, space="PSUM") as s_ps,
            tc.tile_pool(name="o_ps", bufs=1, space="PSUM") as o_ps,
            tc.tile_pool(name="pwork", bufs=6) as pwork,
            tc.tile_pool(name="owork", bufs=2) as owork,
        ):
            # ---- weights / biases first (small), then x half-tiles in
            # consumption order so chunk-0 projections start early ----
            ident = const.tile([P, P], F32, name="ident")
            make_identity(nc, ident)

            w1_sb = const.tile([P, DT, P], F16, name="w1_sb")
            nc.sync.dma_start(out=w1_sb[:], in_=w1.rearrange("(n p) h -> p n h", p=P))

            # xT as 4 quarter-T loads (1MB each == one q-chunk's needs), in
            # consumption order; chunk-0 projections gate on just w1 + xq0.
            # b1 lands before xq1 so the chunk-0 bias-add isn't held back.
            xr = xT.rearrange("(n p) t -> p n t", p=P)
            xq = []
            for ci in range(NCH):
                t = const.tile([P, DT, CH], F16, name=f"xq{ci}")
                nc.sync.dma_start(out=t[:], in_=xr[:, :, ci * CH:(ci + 1) * CH])
                xq.append(t)
                if ci == 0:
                    b1_sb = const.tile([P, 1], F32, name="b1_sb")
                    nc.sync.dma_start(out=b1_sb[:], in_=b1[:, :])
                    wv_sb = const.tile([P, DT, HD], F16, name="wv_sb")
                    nc.sync.dma_start(
                        out=wv_sb[:], in_=wv.rearrange("(n p) h -> p n h", p=P))
                    bv_sb = const.tile([P, 1], F32, name="bv_sb")
                    nc.sync.dma_start(out=bv_sb[:], in_=bv[:, :])


            # HAM warm-up: throwaway matmuls on a memset scratch tile (ready
            # ~4us before any DMA lands) keep the PE busy through its 3.4us
            # activity window, so every real matmul runs at the full 2.4 GHz
            # clock. 18 of them end ~2us before the first real group starts.
            wscr = const.tile([P, CH], F16, name="wscr")
            nc.vector.memset(wscr[:], 0.0)
            for wu in range(18):
                pswu = proj_ps.tile([P, CH], F32, name="warm", tag="proj")
                nc.tensor.matmul(pswu[:], wscr[:, 0:P], wscr[:],
                                 start=True, stop=True)

            # ---- activations ----
            # qk1: rows 0-63 = Q^T, rows 64-127 = K^T; qk2: swapped halves.
            qk1 = acts.tile([P, T], F16, name="qk1")
            qk2 = acts.tile([P, T], F16, name="qk2")
            vT = acts.tile([HD, T], F32, name="vT")
            v_aug = acts.tile([P, NKT, HD + 1], F16, name="v_aug")
            nc.vector.memset(v_aug[:, :, HD], 1.0)

            def qk_chunk(ci):
                cs = slice(ci * CH, (ci + 1) * CH)
                rhs = xq[ci]
                ps = proj_ps.tile([P, CH], F32, name="proj", tag="proj")
                for d in range(DT):
                    nc.tensor.matmul(ps[:], w1_sb[:, d, :], rhs[:, d, :],
                                     start=(d == 0), stop=(d == DT - 1))
                nc.vector.tensor_scalar_add(qk1[:, cs], ps[:], b1_sb[:])
                # half-swapped copy: qk2 = [K^T; Q^T]. 64-partition DVE ops
                # read any aligned src half and write either dest half.
                nc.vector.tensor_copy(qk2[0:HD, cs], qk1[HD:P, cs])
                nc.vector.tensor_copy(qk2[HD:P, cs], qk1[0:HD, cs])

            def v_pair(ca, cb):
                # V projections for two chunks col-packed: chunk ca on array
                # columns 0-63, chunk cb on columns 64-127 -> the matmul pairs
                # overlap in the PE array; outputs land in disjoint halves of
                # one PSUM bank.
                psv = proj_ps.tile([P, CH], F32, name="projv", tag="proj")
                for d in range(DT):
                    nc.tensor.matmul(psv[0:HD, :], wv_sb[:, d, :], xq[ca][:, d, :],
                                     start=(d == 0), stop=(d == DT - 1))
                    nc.tensor.matmul(psv[HD:P, :], wv_sb[:, d, :], xq[cb][:, d, :],
                                     start=(d == 0), stop=(d == DT - 1))
                nc.vector.tensor_scalar_add(
                    vT[:, ca * CH:(ca + 1) * CH], psv[0:HD, :], bv_sb[0:HD])
                nc.vector.tensor_scalar_add(
                    vT[:, cb * CH:(cb + 1) * CH], psv[HD:P, :], bv_sb[HD:P])
                for tt in range(4 * ca, 4 * ca + 8):
                    tp = tr_ps.tile([P, HD], F32, name="vtr", tag="vtr")
                    nc.tensor.transpose(tp[:], vT[:, tt * P:(tt + 1) * P],
                                        ident[:HD, :HD])
                    nc.vector.tensor_copy(v_aug[:, tt, 0:HD], tp[:])

            def attn_chunk(ci):
                cs = slice(ci * CH, (ci + 1) * CH)
                nkt = 4 * (ci + 1)
                ops = o_ps.tile([HD + 1, CH], F32, name="oacc", tag="oacc")
                for j in range(nkt // 2):
                    ka, kb = 2 * j, 2 * j + 1
                    s2 = s_ps.tile([P, 2 * CH], F32, name="sT", tag="sT")
                    # rows 0-63 of the array: K^T from qk2, Q^T from qk1
                    nc.tensor.matmul(s2[:, 0:CH], qk2[0:HD, ka * P:(ka + 1) * P],
                                     qk1[0:HD, cs], start=True, stop=True)
                    # rows 64-127: K^T from qk1, Q^T from qk2 (concurrent)
                    nc.tensor.matmul(s2[:, CH:2 * CH], qk1[HD:P, kb * P:(kb + 1) * P],
                                     qk2[HD:P, cs], start=True, stop=True)
                    pt = pwork.tile([P, 2 * CH], F16, name="pT", tag="pT")
                    nc.scalar.activation(pt[:], s2[:],
                                         mybir.ActivationFunctionType.Exp)
                    # causal mask: zero P^T where key > query (diagonal band)
                    for half, kt in ((0, ka), (1, kb)):
                        delta = kt * P - ci * CH
                        if delta >= 0:
                            nc.gpsimd.affine_select(
                                out=pt[:, half * CH:(half + 1) * CH],
                                in_=pt[:, half * CH:(half + 1) * CH],
                                compare_op=mybir.AluOpType.is_ge, fill=0.0,
                                base=-delta, pattern=[[1, CH]],
                                channel_multiplier=-1,
                            )
                    nc.tensor.matmul(ops[:], v_aug[:, ka, :], pt[:, 0:CH],
                                     start=(j == 0), stop=False)
                    nc.tensor.matmul(ops[:], v_aug[:, kb, :], pt[:, CH:2 * CH],
                                     start=False, stop=(j == nkt // 2 - 1))
                osb = owork.tile([HD + 1, CH], F32, name="osb", tag="osb")
                nc.vector.tensor_copy(osb[:], ops[:])
                nc.sync.dma_start(out=outT[:, cs], in_=osb[:])

            qk_chunk(0)
            v_pair(0, 1)
            attn_chunk(0)
            qk_chunk(1)
            attn_chunk(1)
            qk_chunk(2)
            v_pair(2, 3)
            attn_chunk(2)
            qk_chunk(3)
            attn_chunk(3)

    if legalize:
        _legalize_waits(nc, mybir)
    return nc


def _legalize_waits(nc, mybir):
    """Split multi-wait instructions for the XLA-route walrus codegen.

    The TPB EVENTS struct holds one semaphore wait per instruction and this
    pipeline's codegen refuses >1. Hoist extra waits onto standalone
    EventSemaphore instructions on the same engine queue right before the
    instruction - semantically identical, the queue stalls there.
    """
    n = 0
    for f in nc.m.functions:
        for b in f.blocks:
            out = []
            changed = False
            for inst in b.instructions:
                si = inst.sync_info
                waits = list(si.on_wait) if si is not None and si.on_wait else []
                if len(waits) > 1:
                    changed = True
                    for w in waits[:-1]:
                        n += 1
                        out.append(mybir.InstEventSemaphore(
                            name=f"waitfix{n}_{inst.name}",
                            engine=inst.engine,
                            sync_info=mybir.SyncInfo(on_wait=[w], on_update=[]),
                        ))
                    inst.sync_info = mybir.SyncInfo(
                        on_wait=waits[-1:],
                        on_update=list(si.on_update or []),
                    )
                out.append(inst)
            if changed:
                b.instructions = out
    return n


def kernel(x, wq, bq, wk, bk, wv, bv):
    global LAST_RESULTS
    import os
    os.environ.setdefault("JAX_PLATFORMS", "")
    from concourse.bass_utils import run_bass_kernel_spmd

    x = np.asarray(x, dtype=np.float32)
    s = np.float32(1.0 / np.sqrt(HD))
    wq_s = np.asarray(wq, np.float32) * s
    wk_f = np.asarray(wk, np.float32)
    w1 = np.ascontiguousarray(np.concatenate([wq_s, wk_f], 0).T.astype(np.float16))
    wv_c = np.ascontiguousarray(np.asarray(wv, np.float32).T.astype(np.float16))
    b1 = np.ascontiguousarray(
        np.concatenate([np.asarray(bq, np.float32) * s,
                        np.asarray(bk, np.float32)]).reshape(P, 1))
    bv_f = np.asarray(bv, np.float32)
    bv_c = np.ascontiguousarray(np.concatenate([bv_f, bv_f]).reshape(P, 1))
    xT = np.ascontiguousarray(np.swapaxes(x, 1, 2).astype(np.float16))  # (B, D, T)

    nc = _build_module()
    in_maps = [
        {"xT": xT[b], "w1": w1, "wv": wv_c, "b1": b1, "bv": bv_c}
        for b in range(B)
    ]
    res = None
    for attempt in range(3):
        try:
            res = run_bass_kernel_spmd(nc, in_maps, core_ids=list(range(B)))
            break
        except Exception:
            # transient device wedges (NRT_EXEC_UNIT_UNRECOVERABLE) happen;
            # rebuild the module and retry on a clean execution
            if attempt == 2:
                raise
            nc = _build_module()
    LAST_RESULTS = res

    out = np.empty((B, T, HD), dtype=np.float32)
    for b in range(B):
        oT = res.results[b]["outT"]  # (65, T): rows 0..63 = O^T, row 64 = denom
        out[b] = (oT[:HD] / oT[HD:HD + 1]).T
    return out



# revision 2
# speedup vs baseline: 1.0322x; 1.0322x over previous
"""Trainium2 Bass kernel for a single causal attention head.

Problem: x:(8,2048,1024) f32, per-head projections wq/wk/wv:(64,1024),
biases (64,). Output: softmax(causal(q k^T / sqrt(64))) @ v : (8,2048,64).

Strategy:
  - Data-parallel: batch b -> core b (8 cores, 1 batch each).
  - Host prep packs every input into partition-major, fully contiguous
    per-partition lines so each DMA is ~128 large descriptors:
      * xp:(P, NCH*DT*CH) fp16 - x[b] chunk-major/d-major per partition
        (8KB contiguous per partition per chunk).
      * wall:(P, DT*(P+HD)) fp16 - [wq*s|wk] and wv interleaved per d-tile.
      * bb:(P, 2) f32 - [bq*s;bk] and [bv;bv] columns.
  - Device (per core):
      * qk1 = [wq|wk]^T.T @ x: rows 0-63 = Q^T, rows 64-127 = K^T (PSUM
        accumulate over 8 d-tiles, fp16 matmuls, N=512 chunks).
      * qk2 = half-swapped copy of qk1 -> both Q^T and K^T live on both
        partition halves; scores for two k-tiles share the PE array via
        row packing.
      * vT (64,T) fp16, transposed back to (T,64) tiles via fp16 PE
        transpose, augmented with a ones column (softmax denominator
        rides along the PV matmul).
      * S^T = K^T.T @ Q^T per k-tile; P^T = exp(S^T) on ACT; causal mask
        via gpsimd affine_select restricted to the 128-col diagonal band.
      * Diagonal pairs run FIRST per chunk with column-trimmed scores/
        exp/mask/PV (fully-masked columns never computed); non-diagonal
        pairs follow full-range.
      * O^T_aug[65, T] accumulated in PSUM over k-tiles; row 64 = sum_j P^T.
      * attention for chunk ci emitted right after projection chunk ci.
  - Host post: out[b] = (O^T[0:64] / O^T[64:65]).T  (softmax normalization).
"""

import numpy as np

B, T, D, HD = 8, 2048, 1024, 64
P = 128          # SBUF partitions
CH = 512         # q-chunk (matmul moving dim)
NCH = T // CH    # 4
DT = D // P      # 8 d-tiles
NKT = T // P     # 16 k-tiles
NWARM = 7        # PE clock-ramp warmup matmuls

LAST_RESULTS = None      # BassKernelResults of the most recent run (for test.py)


def _build_module(legalize=True):
    import concourse.bass as bass
    import concourse.mybir as mybir
    from concourse.tile import TileContext

    from concourse.masks import make_identity
    F32 = mybir.dt.float32
    F16 = mybir.dt.float16

    nc = bass.Bass("TRN2", target_bir_lowering=True)

    WCOL = P + HD  # packed weight columns per d-tile: [w1 | wv]
    xp = nc.dram_tensor("xp", (P, NCH * DT * CH), F16, kind="ExternalInput")
    wall = nc.dram_tensor("wall", (P, DT * WCOL), F16, kind="ExternalInput")
    bb = nc.dram_tensor("bb", (P, 2), F32, kind="ExternalInput")
    outT = nc.dram_tensor("outT", (HD + 1, T), F32, kind="ExternalOutput")

    with TileContext(nc) as tc:
        with (
            tc.tile_pool(name="const", bufs=1) as const,
            tc.tile_pool(name="acts", bufs=1) as acts,
            tc.tile_pool(name="proj_ps", bufs=2, space="PSUM") as proj_ps,
            tc.tile_pool(name="tr_ps", bufs=1, space="PSUM") as tr_ps,
            tc.tile_pool(name="s_ps", bufs=2, space="PSUM") as s_ps,
            tc.tile_pool(name="o_ps", bufs=1, space="PSUM") as o_ps,
            tc.tile_pool(name="pwork", bufs=6) as pwork,
            tc.tile_pool(name="owork", bufs=2) as owork,
        ):
            # ---- PE warm-up first: throwaway matmuls keep the PE busy
            # through its clock-ramp window so real matmuls run at full
            # speed. Gated only on the wscr memset, not on any DMA. ----
            wscr = const.tile([P, CH], F16, name="wscr")
            nc.vector.memset(wscr[:], 0.0)
            for wu in range(NWARM):
                pswu = proj_ps.tile([P, CH], F32, name="warm", tag="proj")
                nc.tensor.matmul(pswu[:], wscr[:, 0:P], wscr[:],
                                 start=True, stop=True)

            # ---- input DMAs in consumption order; every transfer is
            # contiguous per partition (large descriptors) ----
            w_sb = const.tile([P, DT * WCOL], F16, name="w_sb")
            nc.sync.dma_start(out=w_sb[:], in_=wall[:, :])
            xq = []
            for ci in range(NCH):
                t = const.tile([P, DT * CH], F16, name=f"xq{ci}")
                nc.sync.dma_start(
                    out=t[:], in_=xp[:, ci * DT * CH:(ci + 1) * DT * CH])
                xq.append(t)
                if ci == 0:
                    b_sb = const.tile([P, 2], F32, name="b_sb")
                    nc.sync.dma_start(out=b_sb[:], in_=bb[:, :])

            ident = const.tile([P, P], F16, name="ident")
            make_identity(nc, ident)

            def w1s(d):
                return w_sb[:, d * WCOL:d * WCOL + P]

            def wvs(d):
                return w_sb[:, d * WCOL + P:(d + 1) * WCOL]

            # ---- activations ----
            # qk1: rows 0-63 = Q^T, rows 64-127 = K^T; qk2: swapped halves.
            qk1 = acts.tile([P, T], F16, name="qk1")
            qk2 = acts.tile([P, T], F16, name="qk2")
            vT = acts.tile([HD, T], F16, name="vT")
            v_aug = acts.tile([P, NKT, HD + 1], F16, name="v_aug")
            nc.vector.memset(v_aug[:, :, HD], 1.0)

            def qk_chunk(ci):
                cs = slice(ci * CH, (ci + 1) * CH)
                rhs = xq[ci]
                ps = proj_ps.tile([P, CH], F32, name="proj", tag="proj")
                for d in range(DT):
                    nc.tensor.matmul(ps[:], w1s(d), rhs[:, d * CH:(d + 1) * CH],
                                     start=(d == 0), stop=(d == DT - 1))
                nc.vector.tensor_scalar_add(qk1[:, cs], ps[:], b_sb[:, 0:1])
                # half-swapped copy: qk2 = [K^T; Q^T]. 64-partition DVE ops
                # read any aligned src half and write either dest half.
                nc.vector.tensor_copy(qk2[0:HD, cs], qk1[HD:P, cs])
                nc.vector.tensor_copy(qk2[HD:P, cs], qk1[0:HD, cs])

            def v_pair(ca, cb):
                # V projections for two chunks col-packed: chunk ca on array
                # columns 0-63, chunk cb on columns 64-127 -> the matmul pairs
                # overlap in the PE array; outputs land in disjoint halves of
                # one PSUM bank.
                psv = proj_ps.tile([P, CH], F32, name="projv", tag="proj")
                for d in range(DT):
                    ds = slice(d * CH, (d + 1) * CH)
                    nc.tensor.matmul(psv[0:HD, :], wvs(d), xq[ca][:, ds],
                                     start=(d == 0), stop=(d == DT - 1))
                    nc.tensor.matmul(psv[HD:P, :], wvs(d), xq[cb][:, ds],
                                     start=(d == 0), stop=(d == DT - 1))
                nc.vector.tensor_scalar_add(
                    vT[:, ca * CH:(ca + 1) * CH], psv[0:HD, :], b_sb[0:HD, 1:2])
                nc.vector.tensor_scalar_add(
                    vT[:, cb * CH:(cb + 1) * CH], psv[HD:P, :], b_sb[HD:P, 1:2])
                for tt in range(4 * ca, 4 * ca + 8):
                    tp = tr_ps.tile([P, HD], F16, name="vtr", tag="vtr")
                    nc.tensor.transpose(tp[:], vT[:, tt * P:(tt + 1) * P],
                                        ident[:HD, :HD])
                    nc.vector.tensor_copy(v_aug[:, tt, 0:HD], tp[:])

            def attn_chunk(ci):
                c0 = ci * CH
                cs = slice(c0, c0 + CH)
                ops = o_ps.tile([HD + 1, CH], F32, name="oacc", tag="oacc")
                # diagonal pairs first (col-trimmed, masked), then full pairs
                pairs = ([(4 * ci, 4 * ci + 1), (4 * ci + 2, 4 * ci + 3)]
                         + [(2 * j, 2 * j + 1) for j in range(2 * ci)])
                for idx, (ka, kb) in enumerate(pairs):
                    da = max(ka * P - c0, 0)  # first unmasked column
                    db = max(kb * P - c0, 0)
                    diag = idx < 2
                    s2 = s_ps.tile([P, 2 * CH], F32, name="sT", tag="sT")
                    # rows 0-63 of the array: K^T from qk2, Q^T from qk1
                    nc.tensor.matmul(s2[:, da:CH],
                                     qk2[0:HD, ka * P:(ka + 1) * P],
                                     qk1[0:HD, c0 + da:c0 + CH],
                                     start=True, stop=True)
                    # rows 64-127: K^T from qk1, Q^T from qk2 (concurrent)
                    nc.tensor.matmul(s2[:, CH + db:2 * CH],
                                     qk1[HD:P, kb * P:(kb + 1) * P],
                                     qk2[HD:P, c0 + db:c0 + CH],
                                     start=True, stop=True)
                    pt = pwork.tile([P, 2 * CH], F16, name="pT", tag="pT")
                    if diag:
                        nc.scalar.activation(pt[:, da:CH], s2[:, da:CH],
                                             mybir.ActivationFunctionType.Exp)
                        nc.scalar.activation(pt[:, CH + db:2 * CH],
                                             s2[:, CH + db:2 * CH],
                                             mybir.ActivationFunctionType.Exp)
                        # causal mask on the 128-col diagonal band only:
                        # keep where (query - delta) >= key  <=>  c' >= p
                        for off in (da, CH + db):
                            nc.gpsimd.affine_select(
                                out=pt[:, off:off + P],
                                in_=pt[:, off:off + P],
                                compare_op=mybir.AluOpType.is_ge, fill=0.0,
                                base=0, pattern=[[1, P]],
                                channel_multiplier=-1,
                            )
                    else:
                        nc.scalar.activation(pt[:], s2[:],
                                             mybir.ActivationFunctionType.Exp)
                    last = idx == len(pairs) - 1
                    nc.tensor.matmul(ops[:, da:CH], v_aug[:, ka, :],
                                     pt[:, da:CH],
                                     start=(idx == 0), stop=False)
                    nc.tensor.matmul(ops[:, db:CH], v_aug[:, kb, :],
                                     pt[:, CH + db:2 * CH],
                                     start=False, stop=last)
                osb = owork.tile([HD + 1, CH], F32, name="osb", tag="osb")
                nc.vector.tensor_copy(osb[:], ops[:])
                nc.sync.dma_start(out=outT[:, cs], in_=osb[:])

            qk_chunk(0)
            v_pair(0, 1)
            attn_chunk(0)
            qk_chunk(1)
            attn_chunk(1)
            qk_chunk(2)
            v_pair(2, 3)
            attn_chunk(2)
            qk_chunk(3)
            attn_chunk(3)

    if legalize:
        _legalize_waits(nc, mybir)
    return nc


def _legalize_waits(nc, mybir):
    """Split multi-wait instructions for the XLA-route walrus codegen.

    The TPB EVENTS struct holds one semaphore wait per instruction and this
    pipeline's codegen refuses >1. Hoist extra waits onto standalone
    EventSemaphore instructions on the same engine queue right before the
    instruction - semantically identical, the queue stalls there.
    """
    n = 0
    for f in nc.m.functions:
        for b in f.blocks:
            out = []
            changed = False
            for inst in b.instructions:
                si = inst.sync_info
                waits = list(si.on_wait) if si is not None and si.on_wait else []
                if len(waits) > 1:
                    changed = True
                    for w in waits[:-1]:
                        n += 1
                        out.append(mybir.InstEventSemaphore(
                            name=f"waitfix{n}_{inst.name}",
                            engine=inst.engine,
                            sync_info=mybir.SyncInfo(on_wait=[w], on_update=[]),
                        ))
                    inst.sync_info = mybir.SyncInfo(
                        on_wait=waits[-1:],
                        on_update=list(si.on_update or []),
                    )
                out.append(inst)
            if changed:
                b.instructions = out
    return n


def kernel(x, wq, bq, wk, bk, wv, bv):
    global LAST_RESULTS
    import os
    os.environ.setdefault("JAX_PLATFORMS", "")
    from concourse.bass_utils import run_bass_kernel_spmd

    x = np.asarray(x, dtype=np.float32)
    s = np.float32(1.0 / np.sqrt(HD))
    WCOL = P + HD
    # wall: per partition p (= row of the D-contraction tile), per d-tile:
    # 128 cols of [wq*s|wk]^T then 64 cols of wv^T.
    w1 = np.concatenate([np.asarray(wq, np.float32) * s,
                         np.asarray(wk, np.float32)], 0).T  # (D, 128)
    wv_t = np.asarray(wv, np.float32).T                      # (D, 64)
    wall = np.concatenate([w1.reshape(DT, P, P),
                           wv_t.reshape(DT, P, HD)], axis=2)  # (DT, P, WCOL)
    wall = np.ascontiguousarray(
        wall.transpose(1, 0, 2).reshape(P, DT * WCOL)).astype(np.float16)
    b1 = np.concatenate([np.asarray(bq, np.float32) * s,
                         np.asarray(bk, np.float32)])
    bv_f = np.asarray(bv, np.float32)
    bb = np.ascontiguousarray(
        np.stack([b1, np.concatenate([bv_f, bv_f])], axis=1))  # (P, 2)
    # xp[b]: partition-major, chunk-major, d-major: row p holds, for each
    # chunk ci and d-tile d, the 512 fp16 values x[b, ci*CH:(ci+1)*CH, d*P+p].
    xp = np.ascontiguousarray(
        x.reshape(B, NCH, CH, DT, P).transpose(0, 4, 1, 3, 2)
        .reshape(B, P, NCH * DT * CH)).astype(np.float16)

    nc = _build_module()
    in_maps = [
        {"xp": xp[b], "wall": wall, "bb": bb}
        for b in range(B)
    ]
    res = None
    for attempt in range(3):
        try:
            res = run_bass_kernel_spmd(nc, in_maps, core_ids=list(range(B)))
            break
        except Exception:
            # transient device wedges (NRT_EXEC_UNIT_UNRECOVERABLE) happen;
            # rebuild the module and retry on a clean execution
            if attempt == 2:
                raise
            nc = _build_module()
    LAST_RESULTS = res

    out = np.empty((B, T, HD), dtype=np.float32)
    for b in range(B):
        oT = res.results[b]["outT"]  # (65, T): rows 0..63 = O^T, row 64 = denom
        out[b] = (oT[:HD] / oT[HD:HD + 1]).T
    return out


# revision 6
# speedup vs baseline: 1.0669x; 1.0337x over previous
"""Trainium2 Bass kernel for a single causal attention head.

Problem: x:(8,2048,1024) f32, per-head projections wq/wk/wv:(64,1024),
biases (64,). Output: softmax(causal(q k^T / sqrt(64))) @ v : (8,2048,64).

Strategy:
  - Data-parallel: batch b -> core b (8 cores, 1 batch each).
  - Host prep packs every input into partition-major, fully contiguous
    per-partition lines so each DMA is ~128 large descriptors:
      * xp:(P, NCH*DT*CH) fp16 - x[b] chunk-major/d-major per partition
        (8KB contiguous per partition per chunk).
      * wall:(P, DT*(P+HD)) fp16 - [wq*s|wk] and wv interleaved per d-tile.
      * bb:(P, 2) f32 - [bq*s;bk] and [bv;bv] columns.
  - Device (per core):
      * qk1 = [wq|wk]^T.T @ x: rows 0-63 = Q^T, rows 64-127 = K^T (PSUM
        accumulate over 8 d-tiles, fp16 matmuls, N=512 chunks).
      * qk2 = half-swapped copy of qk1 -> both Q^T and K^T live on both
        partition halves; scores for two k-tiles share the PE array via
        row packing.
      * vT (64,T) fp16, transposed back to (T,64) tiles via fp16 PE
        transpose, augmented with a ones column (softmax denominator
        rides along the PV matmul).
      * S^T = K^T.T @ Q^T per k-tile; P^T = exp(S^T) on ACT; causal mask
        via gpsimd affine_select restricted to the 128-col diagonal band.
      * Diagonal pairs run FIRST per chunk with column-trimmed scores/
        exp/mask/PV (fully-masked columns never computed); non-diagonal
        pairs follow full-range.
      * O^T_aug[65, T] accumulated in PSUM over k-tiles; row 64 = sum_j P^T.
      * attention for chunk ci emitted right after projection chunk ci.
  - Host post: out[b] = (O^T[0:64] / O^T[64:65]).T  (softmax normalization).
"""

import numpy as np

B, T, D, HD = 8, 2048, 1024, 64
P = 128          # SBUF partitions
CH = 512         # q-chunk (matmul moving dim)
NCH = T // CH    # 4
DT = D // P      # 8 d-tiles
NKT = T // P     # 16 k-tiles
NWARM = 8        # PE clock-ramp warmup matmuls
DHALF = DT // 2  # d-tiles per x half-load (two DMA queues)

LAST_RESULTS = None      # BassKernelResults of the most recent run (for test.py)


def _build_module(legalize=True):
    import concourse.bass as bass
    import concourse.mybir as mybir
    from concourse.tile import TileContext

    from concourse.masks import make_identity
    F32 = mybir.dt.float32
    F16 = mybir.dt.float16

    nc = bass.Bass("TRN2", target_bir_lowering=True)

    WCOL = P + HD  # packed weight columns per d-tile: [w1 | wv]
    xp = nc.dram_tensor("xp", (P, NCH * DT * CH), F16, kind="ExternalInput")
    wall = nc.dram_tensor("wall", (P, DT * WCOL), F16, kind="ExternalInput")
    bb = nc.dram_tensor("bb", (P, 2), F32, kind="ExternalInput")
    outT = nc.dram_tensor("outT", (HD + 1, T), F32, kind="ExternalOutput")

    with TileContext(nc) as tc:
        with (
            tc.tile_pool(name="const", bufs=1) as const,
            tc.tile_pool(name="acts", bufs=1) as acts,
            tc.tile_pool(name="proj_ps", bufs=2, space="PSUM") as proj_ps,
            tc.tile_pool(name="tr_ps", bufs=1, space="PSUM") as tr_ps,
            tc.tile_pool(name="s_ps", bufs=2, space="PSUM") as s_ps,
            tc.tile_pool(name="o_ps", bufs=1, space="PSUM") as o_ps,
            tc.tile_pool(name="pwork", bufs=6) as pwork,
            tc.tile_pool(name="owork", bufs=2) as owork,
        ):
            # ---- PE warm-up first: throwaway matmuls keep the PE busy
            # through its clock-ramp window so real matmuls run at full
            # speed. Gated only on the wscr memset (gpsimd frees earliest),
            # not on any DMA. Any PE idle gap resets the clock ramp, so the
            # warmup count is sized to bridge until the first x half lands.
            wscr = const.tile([P, CH], F16, name="wscr")
            nc.gpsimd.memset(wscr[:], 0.0)
            for wu in range(NWARM):
                pswu = proj_ps.tile([P, CH], F32, name="warm", tag="proj")
                nc.tensor.matmul(pswu[:], wscr[:, 0:P], wscr[:],
                                 start=True, stop=True)

            # ---- input DMAs split across BOTH hardware DGE queues (sync=SP
            # and scalar=ACT are the two HWDGE engines): x chunk halves load
            # in parallel, and projections gate on half `a` only. Every
            # transfer is contiguous per partition (large descriptors). ----
            HB = DHALF * CH  # fp16 elems per half-chunk per partition
            w_sb = const.tile([P, DT * WCOL], F16, name="w_sb")
            nc.scalar.dma_start(out=w_sb[:], in_=wall[:, :])
            xq = []
            for ci in range(NCH):
                ta = const.tile([P, HB], F16, name=f"xq{ci}a")
                tb = const.tile([P, HB], F16, name=f"xq{ci}b")
                base = ci * DT * CH
                nc.sync.dma_start(out=ta[:], in_=xp[:, base:base + HB])
                nc.scalar.dma_start(
                    out=tb[:], in_=xp[:, base + HB:base + 2 * HB])
                xq.append((ta, tb))
                if ci == 0:
                    b_sb = const.tile([P, 2], F32, name="b_sb")
                    nc.scalar.dma_start(out=b_sb[:], in_=bb[:, :])

            def xqs(ci, d):
                ta, tb = xq[ci]
                t = ta if d < DHALF else tb
                dd = d % DHALF
                return t[:, dd * CH:(dd + 1) * CH]

            ident = const.tile([P, P], F16, name="ident")
            make_identity(nc, ident)

            def w1s(d):
                return w_sb[:, d * WCOL:d * WCOL + P]

            def wvs(d):
                return w_sb[:, d * WCOL + P:(d + 1) * WCOL]

            # ---- activations ----
            # qk1: rows 0-63 = Q^T, rows 64-127 = K^T; qk2: swapped halves.
            qk1 = acts.tile([P, T], F16, name="qk1")
            qk2 = acts.tile([P, T], F16, name="qk2")
            vT = acts.tile([HD, T], F16, name="vT")
            v_aug = acts.tile([P, NKT, HD + 1], F16, name="v_aug")
            nc.vector.memset(v_aug[:, :, HD], 1.0)

            def qk_chunk(ci):
                cs = slice(ci * CH, (ci + 1) * CH)
                ps = proj_ps.tile([P, CH], F32, name="proj", tag="proj")
                for d in range(DT):
                    nc.tensor.matmul(ps[:], w1s(d), xqs(ci, d),
                                     start=(d == 0), stop=(d == DT - 1))
                nc.vector.tensor_scalar_add(qk1[:, cs], ps[:], b_sb[:, 0:1])
                # half-swapped copy: qk2 = [K^T; Q^T]. 64-partition DVE ops
                # read any aligned src half and write either dest half.
                nc.vector.tensor_copy(qk2[0:HD, cs], qk1[HD:P, cs])
                nc.vector.tensor_copy(qk2[HD:P, cs], qk1[0:HD, cs])

            def v_pair(ca, cb):
                # V projections for two chunks col-packed: chunk ca on array
                # columns 0-63, chunk cb on columns 64-127 -> the matmul pairs
                # overlap in the PE array; outputs land in disjoint halves of
                # one PSUM bank.
                psv = proj_ps.tile([P, CH], F32, name="projv", tag="proj")
                for d in range(DT):
                    nc.tensor.matmul(psv[0:HD, :], wvs(d), xqs(ca, d),
                                     start=(d == 0), stop=(d == DT - 1))
                    nc.tensor.matmul(psv[HD:P, :], wvs(d), xqs(cb, d),
                                     start=(d == 0), stop=(d == DT - 1))
                nc.vector.tensor_scalar_add(
                    vT[:, ca * CH:(ca + 1) * CH], psv[0:HD, :], b_sb[0:HD, 1:2])
                nc.vector.tensor_scalar_add(
                    vT[:, cb * CH:(cb + 1) * CH], psv[HD:P, :], b_sb[HD:P, 1:2])
                for tt in range(4 * ca, 4 * ca + 8):
                    tp = tr_ps.tile([P, HD], F16, name="vtr", tag="vtr")
                    nc.tensor.transpose(tp[:], vT[:, tt * P:(tt + 1) * P],
                                        ident[:HD, :HD])
                    nc.vector.tensor_copy(v_aug[:, tt, 0:HD], tp[:])

            def chunk_pairs(ci):
                # diagonal pairs first (col-trimmed, masked), then full pairs
                return ([(4 * ci, 4 * ci + 1), (4 * ci + 2, 4 * ci + 3)]
                        + [(2 * j, 2 * j + 1) for j in range(2 * ci)])

            def scores_pair(ci, ka, kb, diag):
                c0 = ci * CH
                da = max(ka * P - c0, 0)  # first unmasked column
                db = max(kb * P - c0, 0)
                s2 = s_ps.tile([P, 2 * CH], F32, name="sT", tag="sT")
                # rows 0-63 of the array: K^T from qk2, Q^T from qk1
                nc.tensor.matmul(s2[:, da:CH],
                                 qk2[0:HD, ka * P:(ka + 1) * P],
                                 qk1[0:HD, c0 + da:c0 + CH],
                                 start=True, stop=True)
                # rows 64-127: K^T from qk1, Q^T from qk2 (concurrent)
                nc.tensor.matmul(s2[:, CH + db:2 * CH],
                                 qk1[HD:P, kb * P:(kb + 1) * P],
                                 qk2[HD:P, c0 + db:c0 + CH],
                                 start=True, stop=True)
                pt = pwork.tile([P, 2 * CH], F16, name="pT", tag="pT")
                if diag:
                    nc.scalar.activation(pt[:, da:CH], s2[:, da:CH],
                                         mybir.ActivationFunctionType.Exp)
                    nc.scalar.activation(pt[:, CH + db:2 * CH],
                                         s2[:, CH + db:2 * CH],
                                         mybir.ActivationFunctionType.Exp)
                    # causal mask on the 128-col diagonal band only:
                    # keep where (query - delta) >= key  <=>  c' >= p
                    for off in (da, CH + db):
                        nc.gpsimd.affine_select(
                            out=pt[:, off:off + P],
                            in_=pt[:, off:off + P],
                            compare_op=mybir.AluOpType.is_ge, fill=0.0,
                            base=0, pattern=[[1, P]],
                            channel_multiplier=-1,
                        )
                else:
                    nc.scalar.activation(pt[:], s2[:],
                                         mybir.ActivationFunctionType.Exp)
                return pt

            def pv_pair(ci, ops, ka, kb, pt, first, last):
                c0 = ci * CH
                da = max(ka * P - c0, 0)
                db = max(kb * P - c0, 0)
                nc.tensor.matmul(ops[:, da:CH], v_aug[:, ka, :],
                                 pt[:, da:CH],
                                 start=first, stop=False)
                nc.tensor.matmul(ops[:, db:CH], v_aug[:, kb, :],
                                 pt[:, CH + db:2 * CH],
                                 start=False, stop=last)

            def store_chunk(ci, ops):
                osb = owork.tile([HD + 1, CH], F32, name="osb", tag="osb")
                nc.vector.tensor_copy(osb[:], ops[:])
                nc.sync.dma_start(
                    out=outT[:, ci * CH:(ci + 1) * CH], in_=osb[:])

            def attn_chunk(ci):
                ops = o_ps.tile([HD + 1, CH], F32, name="oacc", tag="oacc")
                pairs = chunk_pairs(ci)
                for idx, (ka, kb) in enumerate(pairs):
                    pt = scores_pair(ci, ka, kb, diag=idx < 2)
                    pv_pair(ci, ops, ka, kb, pt,
                            first=idx == 0, last=idx == len(pairs) - 1)
                store_chunk(ci, ops)

            qk_chunk(0)
            # chunk 0: scores+exp run before the V projection so the ACT
            # engine (the global long pole) starts ~5us earlier; PV follows
            # once v_aug is ready.
            p0 = chunk_pairs(0)
            pts = [scores_pair(0, ka, kb, diag=True) for ka, kb in p0]
            v_pair(0, 1)
            ops0 = o_ps.tile([HD + 1, CH], F32, name="oacc", tag="oacc")
            for idx, (ka, kb) in enumerate(p0):
                pv_pair(0, ops0, ka, kb, pts[idx],
                        first=idx == 0, last=idx == len(p0) - 1)
            store_chunk(0, ops0)
            qk_chunk(1)
            attn_chunk(1)
            qk_chunk(2)
            v_pair(2, 3)
            attn_chunk(2)
            qk_chunk(3)
            attn_chunk(3)

    if legalize:
        _legalize_waits(nc, mybir)
    return nc


def _legalize_waits(nc, mybir):
    """Split multi-wait instructions for the XLA-route walrus codegen.

    The TPB EVENTS struct holds one semaphore wait per instruction and this
    pipeline's codegen refuses >1. Hoist extra waits onto standalone
    EventSemaphore instructions on the same engine queue right before the
    instruction - semantically identical, the queue stalls there.
    """
    n = 0
    for f in nc.m.functions:
        for b in f.blocks:
            out = []
            changed = False
            for inst in b.instructions:
                si = inst.sync_info
                waits = list(si.on_wait) if si is not None and si.on_wait else []
                if len(waits) > 1:
                    changed = True
                    for w in waits[:-1]:
                        n += 1
                        out.append(mybir.InstEventSemaphore(
                            name=f"waitfix{n}_{inst.name}",
                            engine=inst.engine,
                            sync_info=mybir.SyncInfo(on_wait=[w], on_update=[]),
                        ))
                    inst.sync_info = mybir.SyncInfo(
                        on_wait=waits[-1:],
                        on_update=list(si.on_update or []),
                    )
                out.append(inst)
            if changed:
                b.instructions = out
    return n


def kernel(x, wq, bq, wk, bk, wv, bv):
    global LAST_RESULTS
    import os
    os.environ.setdefault("JAX_PLATFORMS", "")
    from concourse.bass_utils import run_bass_kernel_spmd

    x = np.asarray(x, dtype=np.float32)
    s = np.float32(1.0 / np.sqrt(HD))
    WCOL = P + HD
    # wall: per partition p (= row of the D-contraction tile), per d-tile:
    # 128 cols of [wq*s|wk]^T then 64 cols of wv^T.
    w1 = np.concatenate([np.asarray(wq, np.float32) * s,
                         np.asarray(wk, np.float32)], 0).T  # (D, 128)
    wv_t = np.asarray(wv, np.float32).T                      # (D, 64)
    wall = np.concatenate([w1.reshape(DT, P, P),
                           wv_t.reshape(DT, P, HD)], axis=2)  # (DT, P, WCOL)
    wall = np.ascontiguousarray(
        wall.transpose(1, 0, 2).reshape(P, DT * WCOL)).astype(np.float16)
    b1 = np.concatenate([np.asarray(bq, np.float32) * s,
                         np.asarray(bk, np.float32)])
    bv_f = np.asarray(bv, np.float32)
    bb = np.ascontiguousarray(
        np.stack([b1, np.concatenate([bv_f, bv_f])], axis=1))  # (P, 2)
    # xp[b]: partition-major, chunk-major, d-major: row p holds, for each
    # chunk ci and d-tile d, the 512 fp16 values x[b, ci*CH:(ci+1)*CH, d*P+p].
    xp = np.ascontiguousarray(
        x.reshape(B, NCH, CH, DT, P).transpose(0, 4, 1, 3, 2)
        .reshape(B, P, NCH * DT * CH)).astype(np.float16)

    nc = _build_module()
    in_maps = [
        {"xp": xp[b], "wall": wall, "bb": bb}
        for b in range(B)
    ]
    res = None
    for attempt in range(3):
        try:
            res = run_bass_kernel_spmd(nc, in_maps, core_ids=list(range(B)))
            break
        except Exception:
            # transient device wedges (NRT_EXEC_UNIT_UNRECOVERABLE) happen;
            # rebuild the module and retry on a clean execution
            if attempt == 2:
                raise
            nc = _build_module()
    LAST_RESULTS = res

    out = np.empty((B, T, HD), dtype=np.float32)
    for b in range(B):
        oT = res.results[b]["outT"]  # (65, T): rows 0..63 = O^T, row 64 = denom
        out[b] = (oT[:HD] / oT[HD:HD + 1]).T
    return out
